# revision 61
# baseline (speedup 1.0000x reference)
"""Trainium2 Bass kernel for sliding-window GQA attention (qk-norm + RoPE).

Problem: B=2, S=2048, D=1024, 16 heads / 4 kv heads, head_dim 64,
causal sliding window 512, fp32 I/O.

Sharding: 8 cores = batch(2) x sequence(4). Each core computes 512 query
tokens against a 1024-token context window (512-token halo; chunk 0 is
zero-padded on the left). Fully data-parallel SPMD - no collectives.

v2 design notes (all aimed at PE-instruction count + engine balance):
  - scores: per (head, k-tile) ONE wide matmul (kT stationary, rhs = all
    valid q columns, 128..512 wide) into a bank-packed PSUM slab; 8
    matmuls/head vs 28 in v1.
  - the two sliding-window triangle masks are multiplicative 0/1 tiles
    applied to the exp output on GpSimd; the bank packing puts all four
    triangle-A blocks (and all four B blocks) at uniform stride 640 so
    each set is ONE strided tensor_mul.
  - attn@v: per (head, k-tile) 1-2 wide matmuls with per-column-region
    start/stop accumulate flags (14/head vs 40 in v1); the extra "ones"
    column of v gives the softmax denominator as in v1.
  - denominator: reciprocal read directly from the PSUM row, broadcast
    to 64 partitions with a K=1 f32r ones-outer-product (no DMAs).
  - qk-norm: square + segmented tensor_reduce + Sqrt activation with
    fused scale/bias + reciprocal; one op each per 128-token tile.
  - scalar engine does only: copies in the projection phases (where it
    is otherwise idle), Sqrt (once per tile), then exclusively Exp in
    the attention phase (one act-table switch in the whole kernel).
  - y is written to HBM in bf16 (halves the output DMA); host casts up.

Head-slot permutation: q heads are permuted on the host so every head's
64 q-rows sit at the same SBUF partition offset (0 or 64) as its kv
group's k-rows - matmul requires lhsT/rhs base partitions to match.
wo rows are permuted to match. The within-head dims of q/k are permuted
evens-first so RoPE becomes two contiguous 32-wide halves (scores are
invariant to a shared q/k dim permutation).
"""

import sys

sys.path.insert(0, "/opt/trn_rl_repo")

from contextlib import ExitStack

import numpy as np
import ml_dtypes

import bass_rust
import concourse.bass as bass
import concourse.tile as tile
from concourse import mybir

# ---------------- problem constants ----------------
B, S, D = 2, 2048, 1024
H, KV, HD = 16, 4, 64
WINDOW = 512
EPS = 1e-5
NCORES = 8
TQ = 512          # query tokens per core
TC = 1024         # context tokens per core (incl. 512 halo)
NQT = TQ // 128   # 4 query tiles
NCT = TC // 128   # 8 context tiles
P = 128

F32 = mybir.dt.float32
BF16 = mybir.dt.bfloat16
F32R = mybir.dt.float32r
ALU = mybir.AluOpType
ACTF = mybir.ActivationFunctionType

# q-head -> slot permutation with parity matching:
# slot p must satisfy p%2 == (head//4)%2 so that the q rows (at partition
# offset (p%2)*64) align with the kv group's k rows.
HEAD_OF_SLOT = [0, 4, 1, 5, 2, 6, 3, 7, 8, 12, 9, 13, 10, 14, 11, 15]

# Packed score-slab layout: per head the 8 k-tiles' valid q-ranges
# (widths 128,256,384,512,512,384,256,128; q-starts 0,0,0,0,0,128,256,384)
# are packed into 5 PSUM banks of 512 f32 so that
#   - no k-tile's range crosses a bank boundary, and
#   - the 4 triangle-A blocks sit at flat offsets 0,640,1280,1920 and the
#     4 triangle-B blocks at 128,768,1408,2048 (both stride 640), letting
#     one strided op mask each set.
PACK_W = [128, 256, 384, 512, 512, 384, 256, 128]
QSTART = [0, 0, 0, 0, 0, 128, 256, 384]
PACK_OFF = [0, 512, 1024, 1536, 2048, 128, 768, 1408]
BANK_OF = [o // 512 for o in PACK_OFF]   # 0,1,2,3,4,0,1,2
LOCAL_OF = [o % 512 for o in PACK_OFF]   # 0,0,0,0,0,128,256,384


def split_multiwaits(nc):
    """This environment's walrus build rejects any instruction with more
    than one sync-wait condition. Split extras into preceding single-wait
    NoOps on the same engine (identical blocking semantics)."""
    n_split = 0
    for f in nc.m.functions:
        for blk in f.blocks:
            out = []
            changed = False
            for inst in blk.instructions:
                try:
                    si = inst.sync_info
                    waits = list(si.on_wait)
                except Exception:
                    out.append(inst)
                    continue
                if len(waits) > 1:
                    changed = True
                    for j, w in enumerate(waits[:-1]):
                        nop = mybir.InstNoOp(
                            name=f"{inst.name}-wsplit{j}", ins=[], outs=[])
                        nop.engine = inst.engine
                        nop.sync_info = bass_rust.SyncInfo(
                            on_wait=[w], on_update=[])
                        nc.register_instruction(nop, overwrite=True)
                        out.append(nop)
                        n_split += 1
                    inst.sync_info = bass_rust.SyncInfo(
                        on_wait=[waits[-1]], on_update=list(si.on_update))
                out.append(inst)
            if changed:
                blk.instructions = out
    return n_split


# ---------------- program builder ----------------

def emit(nc, tc, ctx, stop_after="full"):
    cp = ctx.enter_context(tc.tile_pool(name="const", bufs=1))
    scr = ctx.enter_context(tc.tile_pool(name="scr", bufs=3))
    epool = ctx.enter_context(tc.tile_pool(name="epool", bufs=4))
    rpool = ctx.enter_context(tc.tile_pool(name="rpool", bufs=3))

    # DRAM params
    xt_d = nc.declare_dram_parameter("xt", [D, TC], BF16, isOutput=False)
    wq_d = nc.declare_dram_parameter("wq", [D, H * HD], BF16, isOutput=False)
    wkv_d = nc.declare_dram_parameter("wkv", [D, 2 * KV * HD], BF16, isOutput=False)
    wo_d = nc.declare_dram_parameter("wo", [H * HD, D], BF16, isOutput=False)
    cosq_d = nc.declare_dram_parameter("cosq", [P, NQT, HD], BF16, isOutput=False)
    sinq_d = nc.declare_dram_parameter("sinq", [P, NQT, HD], BF16, isOutput=False)
    cosk_d = nc.declare_dram_parameter("cosk", [P, NCT, HD], BF16, isOutput=False)
    sink_d = nc.declare_dram_parameter("sink", [P, NCT, HD], BF16, isOutput=False)
    vmask_d = nc.declare_dram_parameter("vmask", [P, NCT], F32, isOutput=False)
    y_d = nc.declare_dram_parameter("y", [TQ, D], BF16, isOutput=True)

    # persistent SBUF
    xt = cp.tile([P, 8, TC], BF16, tag="xt")
    wq = cp.tile([P, 8, 1024], BF16, tag="wq")
    wkv = cp.tile([P, 8, 512], BF16, tag="wkv")
    wo = cp.tile([P, 8, 1024], BF16, tag="wo")
    cosq = cp.tile([P, NQT, HD], BF16, tag="cosq")
    sinq = cp.tile([P, NQT, HD], BF16, tag="sinq")
    cosk = cp.tile([P, NCT, HD], BF16, tag="cosk")
    sink = cp.tile([P, NCT, HD], BF16, tag="sink")
    vmask = cp.tile([P, NCT], F32, tag="vmask")
    qT = cp.tile([P, 8, TQ], BF16, tag="qT")       # [j, jt, a]
    kT = cp.tile([P, 2, TC], BF16, tag="kT")       # [j, jt2, p]
    vA = cp.tile([P, NCT, KV, 65], BF16, tag="vA")  # v | valid-col @64
    vB = cp.tile([P, NCT, KV, 128], BF16, tag="vB")  # zeros | valid@32 | v@64:
    q_raw = cp.tile([P, NQT, 1024], BF16, tag="qraw")
    qrot = cp.tile([P, NQT, 1024], BF16, tag="qrot")
    k_raw = cp.tile([P, NCT, 256], BF16, tag="kraw")
    krot = cp.tile([P, NCT, 256], BF16, tag="krot")
    oT = cp.tile([P, 8, TQ], BF16, tag="oT")
    y_sb = cp.tile([P, NQT, 1024], BF16, tag="ysb")
    ident = cp.tile([P, P], BF16, tag="ident")
    triA = cp.tile([P, P], BF16, tag="triA")
    triB = cp.tile([P, P], BF16, tag="triB")
    onesb = cp.tile([P, P], BF16, tag="onesb")
    ssq_q = cp.tile([P, NQT, H], F32, tag="ssqq")
    ssq_k = cp.tile([P, NCT, KV], F32, tag="ssqk")
    rsb_q = cp.tile([P, NQT, H], BF16, tag="rsbq")
    rsb_k = cp.tile([P, NCT, KV], BF16, tag="rsbk")
    epsc = cp.tile([P, 1], F32, tag="epsc")
    nc.gpsimd.memset(epsc[:], EPS)

    # ---- input DMAs (split across the two HWDGE queues) ----
    # Input DMAs. Ordering matters: the k/v projection needs wkv + the
    # leading xt chunk, so those go first; everything else follows in
    # consumption order (wq/cosq/sinq before the q phase, wo last).
    nc.sync.dma_start(wkv[:], wkv_d.rearrange("(a p) n -> p a n", p=P))
    xt_r = xt_d.rearrange("(a p) t -> p a t", p=P)
    for c4 in range(4):
        nc.sync.dma_start(xt[:, :, c4 * 256:(c4 + 1) * 256],
                          xt_r[:, :, c4 * 256:(c4 + 1) * 256])
    nc.sync.dma_start(vmask[:], vmask_d[:])
    nc.sync.dma_start(cosk[:], cosk_d[:])
    nc.sync.dma_start(sink[:], sink_d[:])
    nc.scalar.dma_start(cosq[:], cosq_d[:])
    nc.scalar.dma_start(sinq[:], sinq_d[:])

    # ---- on-chip constants ----
    # identity for PE transposes
    nc.gpsimd.memset(ident[:], 0.0)
    nc.gpsimd.affine_select(
        out=ident[:], in_=ident[:], compare_op=ALU.not_equal, fill=1.0,
        base=0, pattern=[[-1, P]], channel_multiplier=1)
    # triangle keep-masks on e[x(k-row), y(q-col)] blocks:
    #   A (k-tile = qb, window lower edge): keep x > y
    #   B (k-tile = qb+4, causal edge):     keep x <= y
    nc.gpsimd.memset(triA[:], 1.0)
    nc.gpsimd.affine_select(
        out=triA[:], in_=triA[:], compare_op=ALU.is_gt, fill=0.0,
        base=0, pattern=[[-1, P]], channel_multiplier=1)
    nc.gpsimd.memset(triB[:], 1.0)
    nc.gpsimd.affine_select(
        out=triB[:], in_=triB[:], compare_op=ALU.is_ge, fill=0.0,
        base=0, pattern=[[1, P]], channel_multiplier=-1)
    nc.gpsimd.memset(onesb[:], 1.0)
    # v augmentation fixed columns: the "ones" column is the per-context-tile
    # validity (0 for left-pad tiles on chunk 0) so padded keys contribute
    # nothing to the softmax denominator.
    nc.vector.memset(vB[:], 0.0)
    for g in range(KV):
        nc.vector.tensor_copy(vA[:, :, g, 64:65], vmask[:].unsqueeze(2))
        nc.vector.tensor_copy(vB[:, :, g, 32:33], vmask[:].unsqueeze(2))

    if stop_after != "full":
        nc.gpsimd.memset(y_sb[:], 0.0)

    inv64 = 1.0 / 64.0

    def rmsnorm_rope(raw, rot, nh, ssq, cosT, sinT, it):
        """raw/rot: [P, nt, nh*64] bf16 slabs; process tile `it`."""
        w = nh * HD
        hv = raw[:, it].rearrange("p (h d) -> p h d", h=nh)
        rv = rot[:, it].rearrange("p (h d) -> p h d", h=nh)
        sq = scr.tile([P, 1024], BF16, tag="sq")
        nc.vector.tensor_mul(sq[:, 0:w], raw[:, it], raw[:, it])
        nc.vector.tensor_reduce(
            out=ssq[:, it], in_=sq[:, 0:w].rearrange("p (h d) -> p h d", h=nh),
            axis=mybir.AxisListType.X, op=ALU.add)
        sd = scr.tile([P, H], F32, tag="sd")
        nc.scalar.activation(sd[:, 0:nh], ssq[:, it], ACTF.Sqrt,
                             bias=epsc[:, 0:1], scale=inv64)
        rsb = scr.tile([P, H], BF16, tag="rsb")
        with nc.allow_low_precision(reason="1/rms scale, O(1) values"):
            nc.vector.reciprocal(rsb[:, 0:nh], sd[:, 0:nh])
        nc.vector.tensor_tensor(
            out=hv[:], in0=hv[:],
            in1=rsb[:, 0:nh].unsqueeze(2).broadcast_to([P, nh, HD]),
            op=ALU.mult)
        # rope in 3 ops: rv = hv*cos + swap(hv)*sin_signed, where swap is a
        # negative-stride view exchanging the two 32-wide halves per head and
        # the host folds the rotate-half sign into the first half of sinT.
        cF = cosT[:, it:it + 1, :].broadcast_to([P, nh, HD])
        sF = (sinT[:, it:it + 1, :].rearrange("p a (u v) -> p a u v", u=2)
              .broadcast_to([P, nh, 2, 32]))
        sw = hv.rearrange("p h (u v) -> p h u v", u=2)[:, :, ::-1, :]
        r1 = scr.tile([P, 1024], BF16, tag="r1")
        r2 = scr.tile([P, 1024], BF16, tag="r2")
        w1 = nh * HD
        r1v = r1[:, 0:w1].rearrange("p (h d) -> p h d", h=nh)
        r2v = r2[:, 0:w1].rearrange("p (h u v) -> p h u v", h=nh, u=2)
        nc.vector.tensor_mul(r1v[:], hv[:], cF)
        nc.vector.tensor_mul(r2v[:], sw, sF)
        nc.gpsimd.tensor_tensor(
            out=rv[:], in0=r1v[:],
            in1=r2[:, 0:w1].rearrange("p (h d) -> p h d", h=nh),
            op=ALU.add)

    # ---- k/v + q projections (phase-scoped PSUM pools) ----
    if stop_after == "dma":
        nc.sync.dma_start(y_d.rearrange("(a p) n -> p a n", p=P), y_sb[:])
        return
    with tc.tile_pool(name="pa", bufs=2, space="PSUM") as pa, \
         tc.tile_pool(name="stp", bufs=2, space="PSUM") as stp:
        # k/v: projections + PSUM evacuation, then squared-sums, then one
        # batched rsqrt, then normalize+rope, then transposes — wave order
        # keeps every engine's in-order stream free of cross-tile stalls.
        def kv_tile(ct):
            kvps = pa.tile([P, 512], F32, tag="pa", name=f"kvps{ct}")
            for dt in range(8):
                nc.tensor.matmul(kvps[:], xt[:, dt, ct * P:(ct + 1) * P],
                                 wkv[:, dt], start=(dt == 0), stop=(dt == 7))
            # k -> k_raw; v -> vA (cols 0:64 per group) and vB (cols 64:128)
            nc.scalar.copy(k_raw[:, ct], kvps[:, 0:256])
            nc.scalar.copy(
                vA[:, ct, :, 0:64],
                kvps[:, 256:512].rearrange("p (g d) -> p g d", g=KV))
            nc.scalar.copy(
                vB[:, ct, :, 64:128],
                kvps[:, 256:512].rearrange("p (g d) -> p g d", g=KV))
            rmsnorm_rope(k_raw, krot, KV, ssq_k, cosk, sink, ct)
            for j2 in range(2):
                tp = stp.tile([P, P], BF16, tag="stp", name=f"ktp{ct}_{j2}")
                nc.tensor.transpose(tp[:], krot[:, ct, j2 * P:(j2 + 1) * P],
                                    ident[:])
                nc.scalar.copy(kT[:, j2, ct * P:(ct + 1) * P], tp[:])

        def q_tile(at):
            qps0 = pa.tile([P, 512], F32, tag="pa", name=f"qps0_{at}")
            qps1 = pa.tile([P, 512], F32, tag="pa", name=f"qps1_{at}")
            for dt in range(8):
                lhs = xt[:, dt, TQ + at * P:TQ + (at + 1) * P]
                nc.tensor.matmul(qps0[:], lhs, wq[:, dt, 0:512],
                                 start=(dt == 0), stop=(dt == 7))
                nc.tensor.matmul(qps1[:], lhs, wq[:, dt, 512:1024],
                                 start=(dt == 0), stop=(dt == 7))
            nc.scalar.copy(q_raw[:, at, 0:512], qps0[:])
            nc.scalar.copy(q_raw[:, at, 512:1024], qps1[:])
            rmsnorm_rope(q_raw, qrot, H, ssq_q, cosq, sinq, at)
            for jt in range(8):
                tp = stp.tile([P, P], BF16, tag="stp", name=f"qtp{at}_{jt}")
                nc.tensor.transpose(tp[:], qrot[:, at, jt * P:(jt + 1) * P],
                                    ident[:])
                if jt % 2 == 0:
                    nc.scalar.copy(qT[:, jt, at * P:(at + 1) * P], tp[:])
                else:
                    nc.vector.tensor_copy(qT[:, jt, at * P:(at + 1) * P],
                                          tp[:])

        if stop_after == "kv":
            for ct in range(NCT):
                kv_tile(ct)
        else:
            # interleave kv and q tiles so the DVE/Pool norm+rope work is
            # spread across the whole projection window instead of bunching
            # after the last projection
            kv_tile(0)
            kv_tile(1)
            nc.scalar.dma_start(wq[:], wq_d.rearrange("(a p) n -> p a n", p=P))
            kv_tile(2)
            kv_tile(3)
            q_tile(0)
            kv_tile(4)
            q_tile(1)
            kv_tile(5)
            q_tile(2)
            kv_tile(6)
            q_tile(3)
            kv_tile(7)

    # ---- attention ----
    if stop_after in ("kv", "q"):
        nc.sync.dma_start(y_d.rearrange("(a p) n -> p a n", p=P), y_sb[:])
        return
    nc.scalar.dma_start(wo[:], wo_d.rearrange("(a p) n -> p a n", p=P))
    scale = float(HD) ** -0.5
    attn_sub = {"attn_sc": 1, "attn_tri": 2, "attn_av": 3}.get(stop_after, 4)
    if attn_sub < 4:
        nc.gpsimd.memset(oT[:], 0.0)
    with tc.tile_pool(name="sbp", bufs=8, space="PSUM") as sbp:
        avp = sbp
        state = {}   # p_slot -> (e, nT, off)

        def head_scores(p_slot):
            g = HEAD_OF_SLOT[p_slot] // 4
            off = (p_slot % 2) * 64       # == (g % 2) * 64 by construction
            # scores: one wide matmul per k-tile into the packed 5-bank slab
            banks = [sbp.tile([P, 512], F32, tag="sb", name=f"sb{p_slot}_{_b}")
                     for _b in range(5)]
            for kt in range(8):
                w, qs, b, lo = PACK_W[kt], QSTART[kt], BANK_OF[kt], LOCAL_OF[kt]
                nc.tensor.matmul(
                    banks[b][:, lo:lo + w],
                    kT[off:off + 64, g // 2, kt * P:(kt + 1) * P],
                    qT[off:off + 64, p_slot // 2, qs:qs + w],
                    start=True, stop=True)
            # exp (ACT) per bank into the packed e slab
            e = epool.tile([P, 5, 512], BF16, tag="e", name=f"e{p_slot}")
            ef = e.rearrange("p a b -> p (a b)")
            for b in range(5):
                nc.scalar.activation(e[:, b], banks[b][:], ACTF.Exp,
                                     bias=0.0, scale=scale)
            if attn_sub < 2:
                return
            # triangle masks: one strided op per set (blocks at stride 640)
            e640 = ef.rearrange("p (a b) -> p a b", b=640)
            nc.gpsimd.tensor_mul(
                e640[:, :, 0:128], e640[:, :, 0:128],
                triA[:].unsqueeze(1).broadcast_to([P, 4, P]))
            nc.gpsimd.tensor_mul(
                e640[:, :, 128:256], e640[:, :, 128:256],
                triB[:].unsqueeze(1).broadcast_to([P, 4, P]))
            state[p_slot] = [e, None]

        def head_av(p_slot):
            g = HEAD_OF_SLOT[p_slot] // 4
            off = (p_slot % 2) * 64
            ef = state[p_slot][0].rearrange("p a b -> p (a b)")
            # attn@v (+ denominator column) accumulated over k-tiles. kt3
            # spans all 512 q columns -> it carries the single start=True
            # write for the whole accumulator; every other k-tile
            # accumulates. (Interleaved per-region start flags produced
            # wrong results on HW - a region's start must precede all
            # accumulates bank-wide.)
            nT = avp.tile([P, 512], F32, tag="sb", name=f"nT{p_slot}")
            nrows = 65 if off == 0 else 128
            lhsT_of = ((lambda kt: vA[:, kt, g]) if off == 0
                       else (lambda kt: vB[:, kt, g]))
            for kt in [3] + [k for k in range(8) if k != 3]:
                w, qs, fo = PACK_W[kt], QSTART[kt], PACK_OFF[kt]
                nc.tensor.matmul(
                    nT[0:nrows, qs:qs + w], lhsT_of(kt),
                    ef[:, fo:fo + w],
                    start=(kt == 3), stop=(kt == 7), skip_group_check=True)
            state[p_slot][1] = nT
            # reciprocal early: it is the long-latency input of the rb
            # broadcast matmul two stages later
            drow = 64 if off == 0 else 32
            rec = None
            if attn_sub >= 4:
                rec = rpool.tile([P, 512], BF16, tag="rec", name=f"rec{p_slot}")
                with nc.allow_low_precision(reason="1/den, O(1) relative"):
                    nc.vector.reciprocal(rec[drow:drow + 1, :],
                                         nT[drow:drow + 1, :])
            nc.vector.tensor_copy(oT[off:off + 64, p_slot // 2, :],
                                  nT[off:off + 64, :])
            state[p_slot].append(rec)

        def head_norm(p_slot):
            off = (p_slot % 2) * 64
            drow = 64 if off == 0 else 32
            rec = state[p_slot][2]
            rb = avp.tile([P, 512], F32, tag="sb", name=f"rb{p_slot}")
            nc.tensor.matmul(rb[:], onesb[drow:drow + 1, :],
                             rec[drow:drow + 1, :],
                             start=True, stop=True)
            nc.vector.tensor_mul(oT[off:off + 64, p_slot // 2, :],
                                 oT[off:off + 64, p_slot // 2, :],
                                 rb[off:off + 64, :])
            del state[p_slot]

        # 3-stage software pipeline over heads: scores/exp/tri (h), attn@v +
        # reciprocal (h-1), broadcast+normalize (h-2). Keeps the PE stream
        # free of wait-on-DVE stalls (the rb matmul's reciprocal input is
        # ready long before the PE reaches it).
        for h in range(H + 2):
            if h < H:
                head_scores(h)
            if attn_sub >= 3 and 0 <= h - 1 < H:
                head_av(h - 1)
            if attn_sub >= 4 and 0 <= h - 2 < H:
                head_norm(h - 2)

    # ---- out-projection ----
    if stop_after.startswith("attn"):
        nc.sync.dma_start(y_d.rearrange("(a p) n -> p a n", p=P), y_sb[:])
        return
    y_r = y_d.rearrange("(a p) n -> p a n", p=P)
    with tc.tile_pool(name="pc", bufs=2, space="PSUM") as pc:
        for at in range(NQT):
            for dh in range(2):
                yps = pc.tile([P, 512], F32, tag="pc")
                for jt in range(8):
                    nc.tensor.matmul(
                        yps[:],
                        oT[:, jt, at * P:(at + 1) * P],
                        wo[:, jt, dh * 512:(dh + 1) * 512],
                        start=(jt == 0), stop=(jt == 7))
                nc.vector.tensor_copy(y_sb[:, at, dh * 512:(dh + 1) * 512],
                                      yps[:])
            nc.sync.dma_start(y_r[:, at], y_sb[:, at])


def build_program(loop_n=1, stop_after="full"):
    """loop_n > 1 wraps the whole kernel body in a hardware For_i loop —
    used by test.py to measure per-iteration HW time with the fixed
    dispatch overhead amortized away. The graded path uses loop_n=1.
    stop_after cuts the body early for phase-attribution experiments."""
    nc = bass.Bass()
    with tile.TileContext(nc) as tc:
        with ExitStack() as ctx:
            if loop_n == 1:
                emit(nc, tc, ctx, stop_after)
            else:
                with tc.For_i(0, loop_n, 1,
                              hint_engines=tuple(mybir.ALL_ENGINES)):
                    emit(nc, tc, ctx, stop_after)
    split_multiwaits(nc)
    return nc


_NC = None


def _get_program():
    global _NC
    if _NC is None:
        _NC = build_program()
    return _NC


# ---------------- host-side prep ----------------

def prep_core_inputs(x, wq, wk, wv, wo, qn_w, kn_w):
    bf = ml_dtypes.bfloat16
    perm = np.concatenate([np.arange(0, 64, 2), np.arange(1, 64, 2)])

    wq_p = np.ascontiguousarray(
        wq.reshape(D, H, HD)[:, HEAD_OF_SLOT][:, :, perm].reshape(D, H * HD)
    ).astype(bf)
    wk_p = np.ascontiguousarray(
        wk.reshape(D, KV, HD)[:, :, perm].reshape(D, KV * HD))
    wkv_p = np.concatenate([wk_p, wv], axis=1).astype(bf)
    wo_p = np.ascontiguousarray(
        wo.reshape(H, HD, D)[HEAD_OF_SLOT].reshape(H * HD, D)).astype(bf)

    inv_freq = 1.0 / (10000.0 ** (np.arange(0, HD, 2, dtype=np.float64) / HD))
    freq64 = np.concatenate([inv_freq, inv_freq])  # emb[t, d] = t * freq64[d]

    def rope_tables(tvec, w):
        ang = tvec[:, None].astype(np.float64) * freq64[None, :]
        c = np.cos(ang).astype(np.float32)
        s_ = np.sin(ang).astype(np.float32)
        we, wo_ = w[0::2], w[1::2]
        cosT = np.concatenate([we[None] * c[:, 0::2], wo_[None] * c[:, 1::2]], axis=1)
        # first half carries the rotate-half minus sign (kernel adds both terms)
        sinT = np.concatenate([-(wo_[None] * s_[:, 0::2]), we[None] * s_[:, 1::2]], axis=1)
        return cosT, sinT

    in_maps = []
    for core in range(NCORES):
        b, ci = divmod(core, NCORES // B)
        q_lo = ci * TQ
        c_lo = q_lo - WINDOW
        ctx_blk = np.zeros((TC, D), np.float32)
        lo = max(c_lo, 0)
        ctx_blk[lo - c_lo:, :] = x[b, lo:q_lo + TQ, :]
        xt_c = np.ascontiguousarray(ctx_blk.T).astype(bf)

        tq = np.arange(q_lo, q_lo + TQ)
        cq, sq = rope_tables(tq, qn_w)
        cosq_c = np.ascontiguousarray(
            cq.reshape(NQT, P, HD).transpose(1, 0, 2)).astype(bf)
        sinq_c = np.ascontiguousarray(
            sq.reshape(NQT, P, HD).transpose(1, 0, 2)).astype(bf)
        tk = np.arange(c_lo, c_lo + TC)
        ck, sk = rope_tables(tk, kn_w)
        cosk_c = np.ascontiguousarray(
            ck.reshape(NCT, P, HD).transpose(1, 0, 2)).astype(bf)
        sink_c = np.ascontiguousarray(
            sk.reshape(NCT, P, HD).transpose(1, 0, 2)).astype(bf)

        vm = np.ones((P, NCT), np.float32)
        n_pad_tiles = (lo - c_lo) // P
        vm[:, :n_pad_tiles] = 0.0

        in_maps.append({
            "xt": xt_c, "wq": wq_p, "wkv": wkv_p, "wo": wo_p,
            "cosq": cosq_c, "sinq": sinq_c, "cosk": cosk_c, "sink": sink_c,
            "vmask": vm,
        })
    return in_maps


def kernel(x, wq, wk, wv, wo, qn_w, kn_w):
    from concourse.bass_utils import run_bass_kernel_spmd
    in_maps = prep_core_inputs(x, wq, wk, wv, wo, qn_w, kn_w)
    nc = _get_program()
    res = run_bass_kernel_spmd(nc, in_maps, list(range(NCORES)))
    out = np.empty((B, S, D), np.float32)
    for core in range(NCORES):
        b, ci = divmod(core, NCORES // B)
        out[b, ci * TQ:(ci + 1) * TQ, :] = np.asarray(
            res.results[core]["y"]).astype(np.float32)
    return out
